# revision 1
# baseline (speedup 1.0000x reference)
"""Trainium2 Bass kernel for nn_DNN_89678917141217 (dense_mlp).

Embedding gather + tf-idf mean-pool, 5 dense layers (1024->4096->4096x3->4096),
tiny output head (4 labels) + log_softmax over B=1024, S=128.

Strategy (8 NeuronCores, SPMD single program):
  - Phase 1 data-parallel over batch: core c gathers the 128*128 embedding rows
    for its 128 batch rows via indirect DMA (one [128,1024]-row gather per
    batch row), pools them with "masked scores" matmuls (lhsT = scores column b
    placed in column b%32 of a [128,32] zero tile, accumulated over b in PSUM
    with 32-row col-tiling so all 128 batch rows land on distinct PSUM
    partitions), then PE-transposes pooled to feature-major and AllGathers it.
  - L1..L5 tensor-parallel over the hidden dim: core c owns features
    [512c, 512c+512) of every layer; activations are kept feature-major
    x_T [4096, B] and AllGathered between layers in two batch halves so the
    collective overlaps the matmuls.
  - Output head: partial logits [1024, 4] per core contracted over the local
    512 features, ReduceScatter(sum) routes each core the [128, 4] slice for
    its own batch rows; bias + log_softmax on device; host concatenates.

Matmuls run in float32r (full-rate fp32 storage, tf32-like rounding) or
bfloat16 depending on MLP_DT / EMB_DT below.
"""

import os
import sys

sys.path.insert(0, '/opt/trn_rl_repo')

import numpy as np
import ml_dtypes

import concourse.bass as bass
import concourse.mybir as mybir
import concourse.tile as tile
from concourse import bacc
from concourse.bass_utils import run_bass_kernel_spmd
from concourse.masks import make_identity

F32 = mybir.dt.float32
F32R = mybir.dt.float32r
BF16 = mybir.dt.bfloat16
I32 = mybir.dt.int32

# dtype config: "f32r" or "bf16". bf16 is validated at 1.2e-6 rel error on
# this problem (log_softmax + fp32 PSUM accumulation cancel the quantization
# noise) and halves all DMA traffic.
EMB_MODE = os.environ.get("KERNEL_EMB_DT", "bf16")
MLP_MODE = os.environ.get("KERNEL_MLP_DT", "bf16")
EMB_DT = F32R if EMB_MODE == "f32r" else BF16
MLP_DT = F32R if MLP_MODE == "f32r" else BF16


def _np_of(dt):
    return np.float32 if dt == F32R else ml_dtypes.bfloat16


NC = 8
P = 128
VOCAB = 50257
EMB = 1024
HID = 4096
NLAB = 4
B, S = 1024, 128
BL = B // NC          # local batch rows per core = 128
FS = HID // NC        # feature shard per core = 512
GCH = 8               # stream columns (128 rows each) per dma_gather call
KC = 2                # k-tiles per rhs chunk in the MLP layers
VSPLIT = 32768        # vocab split so indices fit int16
RG = [list(range(NC))]

LAST_RESULTS = None   # BassKernelResults of the last run (for test harness)
_PROGRAM_CACHE = None


def _build_program(loc, hic, sim=False):
    """loc/hic: columns (of 128 stream slots) in the low/high token streams."""
    nc = bacc.Bacc("TRN2", target_bir_lowering=False, debug=False,
                   enable_asserts=False, num_devices=1 if sim else NC)

    def _collective(kind, op, ins, outs):
        """Real collective, or (sim mode) local DMA copies with roughly the
        same local-HBM traffic so TimelineSim sees equivalent load."""
        if not sim:
            nc.gpsimd.collective_compute(kind, op, replica_groups=RG,
                                         ins=ins, outs=outs)
            return
        in_ap, out_ap = ins[0], outs[0]
        if kind == "AllGather":
            n = in_ap.shape[0]
            for r in range(NC):
                nc.sync.dma_start(out_ap[r * n:(r + 1) * n], in_ap[:])
        else:  # ReduceScatter
            n = out_ap.shape[0]
            nc.sync.dma_start(out_ap[:], in_ap[0:n])

    # ---------------- I/O ----------------
    tot = loc + hic
    I16 = mybir.dt.int16
    idx_lo = nc.dram_tensor("idx_lo", [P, loc * 8], I16, kind="ExternalInput")
    idx_hi = nc.dram_tensor("idx_hi", [P, hic * 8], I16, kind="ExternalInput")
    masks = nc.dram_tensor("masks", [P, tot, P], EMB_DT, kind="ExternalInput")
    emb_lo = nc.dram_tensor("emb_lo", [VSPLIT, EMB], EMB_DT,
                            kind="ExternalInput")
    emb_hi = nc.dram_tensor("emb_hi", [VOCAB - VSPLIT, EMB], EMB_DT,
                            kind="ExternalInput")
    w1t = nc.dram_tensor("w1t", [EMB, FS], MLP_DT, kind="ExternalInput")
    wts = [nc.dram_tensor(f"w{k}t", [HID, FS], MLP_DT, kind="ExternalInput")
           for k in range(2, 6)]
    bs = [nc.dram_tensor(f"b{k}", [P, FS // P], F32, kind="ExternalInput")
          for k in range(1, 6)]
    woutt = nc.dram_tensor("woutt", [FS, NLAB], MLP_DT, kind="ExternalInput")
    bout = nc.dram_tensor("bout", [P, NLAB], F32, kind="ExternalInput")
    out_loc = nc.dram_tensor("out_loc", [BL, NLAB], F32, kind="ExternalOutput")

    with tile.TileContext(nc) as tc:
        with tc.tile_pool(name="const", bufs=1) as const, \
             tc.tile_pool(name="op", bufs=2) as op, \
             tc.tile_pool(name="wp", bufs=1 if MLP_DT == F32R else 2) as wp, \
             tc.tile_pool(name="xp", bufs=12) as xp, \
             tc.tile_pool(name="smps", bufs=2, space="PSUM") as smps, \
             tc.tile_pool(name="dram", bufs=1, space="DRAM") as dram:

            # ------------- constants / small loads -------------
            itlo = const.tile([P, loc * 8], I16, name="itlo")
            nc.sync.dma_start(itlo[:], idx_lo[:])
            ithi = const.tile([P, hic * 8], I16, name="ithi")
            nc.sync.dma_start(ithi[:], idx_hi[:])
            ident = const.tile([P, P], F32, name="ident")
            make_identity(nc, ident[:])
            bsb = []
            for k in range(5):
                t = const.tile([P, FS // P], F32, name=f"bsb{k}", tag=f"bsb{k}")
                nc.sync.dma_start(t[:], bs[k][:])
                bsb.append(t)
            woutsb = const.tile([P, FS // P, NLAB], MLP_DT, name="woutsb")
            nc.sync.dma_start(
                woutsb[:], woutt[:].rearrange("(mt p) l -> p mt l", p=P))
            boutsb = const.tile([P, NLAB], F32, name="boutsb")
            nc.sync.dma_start(boutsb[:], bout[:])

            # L1 weights [128, ko=8, m=512]
            w1sb = const.tile([P, EMB // P, FS], MLP_DT, name="w1sb")
            nc.sync.dma_start(w1sb[:], w1t[:].rearrange("(ko p) m -> p ko m", p=P))

            pooled = const.tile([P, EMB], F32, name="pooled")
            pooledT = const.tile([P, EMB // P, P], MLP_DT, name="pooledT")
            x5sb = const.tile([P, FS // P, B], MLP_DT, name="x5sb")
            lg = const.tile([P, B // P, NLAB], F32, name="lg")

            # ============ phase 1 (scoped pools) ============
            # Streams: (loc + hic) columns of 128 gathered rows each; column c
            # is pooled into all batch rows via host-built mask lhsT.
            with tc.tile_pool(name="maskp", bufs=3) as maskp, \
                 tc.tile_pool(name="poolps", bufs=1, space="PSUM") as poolps, \
                 tc.tile_pool(name="gp", bufs=3) as gp:
                pp0 = poolps.tile([P, 512], F32, name="pp0")
                pp1 = poolps.tile([P, 512], F32, name="pp1")

                chunks = []  # (global_col_start, ncols, table_ap, idx_tile)
                for cols, tbl_ap, it_sb, base in (
                        (loc, emb_lo, itlo, 0), (hic, emb_hi, ithi, loc)):
                    for c0 in range(0, cols, GCH):
                        nch = min(GCH, cols - c0)
                        chunks.append((base + c0, c0, nch, tbl_ap, it_sb))

                for gc0, c0, nch, tbl_ap, it_sb in chunks:
                    g = gp.tile([P, GCH, EMB], EMB_DT, name="g", tag="g")
                    mk = maskp.tile([P, GCH, P], EMB_DT, name="mk", tag="mk")
                    nc.sync.dma_start(mk[:, :nch], masks[:, gc0:gc0 + nch, :])
                    nc.gpsimd.dma_gather(
                        out_ap=g[:, :nch], in_ap=tbl_ap[:],
                        idxs_ap=it_sb[:, c0 * 8:(c0 + nch) * 8],
                        num_idxs=nch * P, num_idxs_reg=nch * P,
                        elem_size=EMB)
                    for j in range(nch):
                        c = gc0 + j
                        nc.tensor.matmul(
                            pp0[:], lhsT=mk[:, j], rhs=g[:, j, 0:512],
                            start=(c == 0), stop=(c == tot - 1))
                        nc.tensor.matmul(
                            pp1[:], lhsT=mk[:, j], rhs=g[:, j, 512:EMB],
                            start=(c == 0), stop=(c == tot - 1))

                nc.vector.tensor_copy(pooled[:, 0:512], pp0[:])
                nc.vector.tensor_copy(pooled[:, 512:EMB], pp1[:])

                # transpose pooled -> pooledT [128 e_in, 8 eo, 128 b]
                for eo in range(EMB // P):
                    tp = smps.tile([P, P], F32, name="tp", tag="tp")
                    nc.tensor.transpose(tp[:], pooled[:, eo * P:(eo + 1) * P],
                                        ident[:])
                    nc.vector.tensor_copy(pooledT[:, eo, :], tp[:])

            pooledT_in = dram.tile([EMB, P], MLP_DT, name="pooledT_in",
                                   tag="pooledT_in")
            nc.sync.dma_start(
                pooledT_in[:].rearrange("(eo p) b -> p eo b", p=P), pooledT[:])
            pooledT_full = dram.tile([NC * EMB, P], MLP_DT, name="pooledT_full",
                                     tag="pooledT_full",
                                     addr_space="Local" if sim else "Shared")
            _collective("AllGather", mybir.AluOpType.bypass,
                        [pooledT_in.opt()], [pooledT_full.opt()])

            # AG buffers for L1..L4 outputs
            xag_in = {}
            xag_out = {}
            for l in range(1, 5):
                for h in range(2):
                    xag_in[(l, h)] = dram.tile(
                        [FS, 512], MLP_DT, name=f"xag_in_{l}_{h}",
                        tag=f"xag_in_{l}_{h}")
                    xag_out[(l, h)] = dram.tile(
                        [HID, 512], MLP_DT, name=f"xag_out_{l}_{h}",
                        tag=f"xag_out_{l}_{h}",
                        addr_space="Local" if sim else "Shared")

            # view of pooledT_full as [p, eo, rb, j]
            ptf = pooledT_full[:].rearrange(
                "(rb eo p) j -> p eo rb j", rb=NC, p=P)

            # ============ MLP (scoped pools) ============
            with tc.tile_pool(name="mmps", bufs=6, space="PSUM") as mmps:
                # ------------- L1 -------------
                for h in range(2):
                    xo = op.tile([P, FS // P, 512], MLP_DT, name="xo1", tag="xo")
                    pss = [mmps.tile([P, 512], F32, name=f"mm1_{h}_{m}",
                                     tag="mm") for m in range(FS // P)]
                    nko = EMB // P
                    for kc in range(nko // KC):
                        rc = xp.tile([P, KC, 4, P], MLP_DT, name="rc1", tag="x")
                        for kk in range(KC):
                            nc.sync.dma_start(
                                rc[:, kk], ptf[:, kc * KC + kk,
                                               h * 4:(h + 1) * 4, :])
                        for kk in range(KC):
                            k = kc * KC + kk
                            for m in range(FS // P):
                                nc.tensor.matmul(
                                    pss[m][:],
                                    lhsT=w1sb[:, k, m * P:(m + 1) * P],
                                    rhs=rc[:, kk].rearrange(
                                        "p rb j -> p (rb j)"),
                                    start=(k == 0), stop=(k == nko - 1))
                    for m in range(FS // P):
                        nc.vector.tensor_scalar_add(xo[:, m], pss[m][:],
                                                    bsb[0][:, m:m + 1])
                    nc.sync.dma_start(
                        xag_in[(1, h)][:].rearrange("(mt p) b -> p mt b", p=P),
                        xo[:])
                    _collective("AllGather", mybir.AluOpType.bypass,
                                [xag_in[(1, h)].opt()], [xag_out[(1, h)].opt()])

                # ------------- L2..L5 -------------
                for li, l in enumerate(range(2, 6)):
                    wsb = wp.tile([P, HID // P, FS], MLP_DT, name=f"wsb{l}",
                                  tag="w")
                    wsrc = wts[li][:].rearrange("(ko p) m -> p ko m", p=P)
                    for wc in range(4):
                        nc.sync.dma_start(wsb[:, wc * 8:(wc + 1) * 8, :],
                                          wsrc[:, wc * 8:(wc + 1) * 8, :])
                    nko = HID // P
                    for h in range(2):
                        src = xag_out[(l - 1, h)][:].rearrange(
                            "(ko p) b -> p ko b", p=P)
                        if l < 5:
                            xo = op.tile([P, FS // P, 512], MLP_DT,
                                         name=f"xo{l}", tag="xo")
                        pss = [mmps.tile([P, 512], F32, name=f"mm{l}_{h}_{m}",
                                         tag="mm") for m in range(FS // P)]
                        for kc in range(nko // KC):
                            rc = xp.tile([P, KC, 512], MLP_DT, name=f"rc{l}",
                                         tag="x")
                            nc.sync.dma_start(
                                rc[:], src[:, kc * KC:(kc + 1) * KC, :])
                            for kk in range(KC):
                                k = kc * KC + kk
                                for m in range(FS // P):
                                    nc.tensor.matmul(
                                        pss[m][:],
                                        lhsT=wsb[:, k, m * P:(m + 1) * P],
                                        rhs=rc[:, kk],
                                        start=(k == 0), stop=(k == nko - 1))
                        for m in range(FS // P):
                            if l < 5:
                                nc.vector.tensor_scalar_add(
                                    xo[:, m], pss[m][:], bsb[l - 1][:, m:m + 1])
                            else:
                                nc.vector.tensor_scalar_add(
                                    x5sb[:, m, h * 512:(h + 1) * 512],
                                    pss[m][:], bsb[4][:, m:m + 1])
                        if l < 5:
                            nc.sync.dma_start(
                                xag_in[(l, h)][:].rearrange(
                                    "(mt p) b -> p mt b", p=P), xo[:])
                            _collective("AllGather", mybir.AluOpType.bypass,
                                        [xag_in[(l, h)].opt()],
                                        [xag_out[(l, h)].opt()])

            # ------------- logits partials + RS + log_softmax -------------
            for q in range(B // P):
                pl = smps.tile([P, NLAB], F32, name="pl", tag="tp")
                for mt in range(FS // P):
                    nc.tensor.matmul(
                        pl[:], lhsT=x5sb[:, mt, q * P:(q + 1) * P],
                        rhs=woutsb[:, mt], start=(mt == 0),
                        stop=(mt == FS // P - 1))
                nc.vector.tensor_copy(lg[:, q, :], pl[:])

            rs_in = dram.tile([B, NLAB], F32, name="rs_in", tag="rs_in")
            nc.sync.dma_start(
                rs_in[:].rearrange("(q p) l -> p q l", p=P), lg[:])
            rs_out = dram.tile([BL, NLAB], F32, name="rs_out", tag="rs_out")
            _collective("ReduceScatter", mybir.AluOpType.add,
                        [rs_in.opt()], [rs_out.opt()])

            lgl = const.tile([P, NLAB], F32, name="lgl")
            nc.sync.dma_start(lgl[:], rs_out[:])
            nc.vector.tensor_add(out=lgl[:], in0=lgl[:], in1=boutsb[:])
            negmx = const.tile([P, 1], F32, name="negmx")
            nc.vector.reduce_max(negmx[:], lgl[:], axis=mybir.AxisListType.X)
            nc.vector.tensor_scalar_mul(negmx[:], negmx[:], -1.0)
            ex = const.tile([P, NLAB], F32, name="ex")
            se = const.tile([P, 1], F32, name="se")
            nc.scalar.activation(ex[:], lgl[:], mybir.ActivationFunctionType.Exp,
                                 bias=negmx[:, 0:1], scale=1.0, accum_out=se[:])
            ls = const.tile([P, 1], F32, name="ls")
            nc.scalar.activation(ls[:], se[:], mybir.ActivationFunctionType.Ln)
            osb = const.tile([P, NLAB], F32, name="osb")
            nc.vector.tensor_scalar(osb[:], lgl[:], negmx[:, 0:1], ls[:, 0:1],
                                    mybir.AluOpType.add, mybir.AluOpType.subtract)
            nc.sync.dma_start(out_loc[:], osb[:])

    nc.compile()
    return nc


def get_program(loc, hic):
    global _PROGRAM_CACHE
    if _PROGRAM_CACHE is None or _PROGRAM_CACHE[0] != (loc, hic):
        _PROGRAM_CACHE = ((loc, hic), _build_program(loc, hic))
    return _PROGRAM_CACHE[1]


def _build_streams(sentence, scores):
    """Per-core low/high token streams + mask tensors.

    Returns (loc, hic, per_core list of dicts with idx_lo, idx_hi, masks).
    """
    emb_np = _np_of(EMB_DT)
    lows, highs = [], []
    for c in range(NC):
        sl = slice(c * BL, (c + 1) * BL)
        sent = np.asarray(sentence[sl])
        sc = np.asarray(scores[sl], np.float32) / np.float32(S)
        is_hi = sent >= VSPLIT
        order = np.argsort(is_hi, axis=1, kind="stable")
        toks = np.take_along_axis(sent, order, 1)
        scs = np.take_along_axis(sc, order, 1)
        nlo = (~is_hi).sum(1)
        lo_toks, lo_scs, lo_rows = [], [], []
        hi_toks, hi_scs, hi_rows = [], [], []
        for b in range(BL):
            n = int(nlo[b])
            lo_toks.append(toks[b, :n])
            lo_scs.append(scs[b, :n])
            lo_rows.append(np.full(n, b, np.int32))
            hi_toks.append(toks[b, n:] - VSPLIT)
            hi_scs.append(scs[b, n:])
            hi_rows.append(np.full(S - n, b, np.int32))
        lows.append((np.concatenate(lo_toks), np.concatenate(lo_scs),
                     np.concatenate(lo_rows)))
        highs.append((np.concatenate(hi_toks), np.concatenate(hi_scs),
                      np.concatenate(hi_rows)))

    loc = max((len(t) + P - 1) // P for t, _, _ in lows)
    hic = max((len(t) + P - 1) // P for t, _, _ in highs)
    # round up to multiples of GCH columns? not needed; chunks handle tails.
    per_core = []
    for c in range(NC):
        masks = np.zeros((P, loc + hic, P), np.float32)
        idx_arrs = []
        for (toks, scs, rows), cols, cbase in ((lows[c], loc, 0),
                                               (highs[c], hic, loc)):
            n = len(toks)
            stream = np.zeros(cols * P, np.int16)
            stream[:n] = toks.astype(np.int16)
            pos = np.arange(n)
            masks[pos % P, cbase + pos // P, rows] = scs
            idx = np.tile(stream.reshape(cols * 8, 16).T, (8, 1))
            idx_arrs.append(np.ascontiguousarray(idx.astype(np.int16)))
        per_core.append({
            "idx_lo": idx_arrs[0], "idx_hi": idx_arrs[1],
            "masks": np.ascontiguousarray(masks).astype(emb_np),
        })
    return loc, hic, per_core


def prep_in_maps(sentence, scores, emb, W1, b1, W2, b2, W3, b3, W4, b4, W5,
                 b5, Wout, bout):
    emb_np = _np_of(EMB_DT)
    mlp_np = _np_of(MLP_DT)

    sentence = np.asarray(sentence)
    scores = np.asarray(scores, dtype=np.float32)
    loc, hic, streams = _build_streams(sentence, scores)
    emb_h = np.ascontiguousarray(np.asarray(emb, dtype=np.float32)).astype(
        emb_np, copy=False)
    emb_lo_h = np.ascontiguousarray(emb_h[:VSPLIT])
    emb_hi_h = np.ascontiguousarray(emb_h[VSPLIT:])
    w1t_h = np.ascontiguousarray(np.asarray(W1, np.float32).T).astype(
        mlp_np, copy=False)
    wts_h = [np.ascontiguousarray(np.asarray(w, np.float32).T).astype(
        mlp_np, copy=False) for w in (W2, W3, W4, W5)]
    woutt_h = np.ascontiguousarray(np.asarray(Wout, np.float32).T).astype(
        mlp_np, copy=False)
    bss = [np.asarray(b, np.float32) for b in (b1, b2, b3, b4, b5)]
    bout_h = np.tile(np.asarray(bout, np.float32)[None, :], (P, 1))

    in_maps = []
    for c in range(NC):
        m = {
            "emb_lo": emb_lo_h,
            "emb_hi": emb_hi_h,
            "w1t": np.ascontiguousarray(w1t_h[:, c * FS:(c + 1) * FS]),
            "woutt": np.ascontiguousarray(woutt_h[c * FS:(c + 1) * FS]),
            "bout": bout_h,
        }
        m.update(streams[c])
        for k in range(2, 6):
            m[f"w{k}t"] = np.ascontiguousarray(
                wts_h[k - 2][:, c * FS:(c + 1) * FS])
        for k in range(1, 6):
            m[f"b{k}"] = np.ascontiguousarray(
                bss[k - 1][c * FS:(c + 1) * FS].reshape(FS // P, P).T)
        in_maps.append(m)
    return (loc, hic), in_maps


def kernel(sentence, scores, emb, W1, b1, W2, b2, W3, b3, W4, b4, W5, b5,
           Wout, bout):
    global LAST_RESULTS
    (loc, hic), in_maps = prep_in_maps(sentence, scores, emb, W1, b1, W2, b2,
                                       W3, b3, W4, b4, W5, b5, Wout, bout)
    nc = get_program(loc, hic)
    res = run_bass_kernel_spmd(nc, in_maps, core_ids=list(range(NC)))
    LAST_RESULTS = res
    out = np.concatenate([res.results[c]["out_loc"] for c in range(NC)], axis=0)
    return out.astype(np.float32)



# revision 5
# speedup vs baseline: 1.4411x; 1.4411x over previous
"""Trainium2 Bass kernel for nn_DNN_89678917141217 (dense_mlp).

Embedding gather + tf-idf mean-pool, 5 dense layers (1024->4096->4096x3->4096),
tiny output head (4 labels) + log_softmax over B=1024, S=128.

Strategy (8 NeuronCores, SPMD, DP=2 x TP=4 hybrid, fp8 e4m3 throughout):
  - Cores are split into 2 data-parallel groups of 4 ({0-3}: batch rows
    0-511, {4-7}: rows 512-1023). Within a group, the hidden dim is
    tensor-parallel 4-way (1024 features per core per layer).
  - Phase 1 per core: dedup the 16384 tokens of its own 128 batch rows,
    indirect-DMA-gather the distinct embedding rows (fp8 table, scaled 2^6),
    pool them into [128 batch, 1024 emb] with host-built score-mask matmuls
    (fp8 DoubleRow, two stream columns per instruction), PE-transpose to
    feature-major and AllGather within the group.
  - All matmuls run in fp8 (e4m3) DoubleRow mode: 2 k-tiles of 128
    contracted per instruction at 0.5 cycles/row (2x bf16 throughput).
    Power-of-2 scales keep tensors in e4m3 range and are folded out
    exactly at each fp32 PSUM drain; validated at 1.7e-5 rel error.
  - L2..L5: weights stay resident in SBUF; activations x_T [4096, 512]
    fp8 are AllGathered in two 2048-feature chunks so the collective of
    chunk A overlaps the matmuls of chunk B (and the next layer starts
    on chunk A while chunk B is still in flight). Weight input dims are
    host-permuted to match the chunked AG row order.
  - Head: fp8 partial logits contracted over local 1024 features,
    ReduceScatter(sum) within the group routes each core its own 128
    rows; bias + log_softmax on device; host concatenates.
"""

import sys

sys.path.insert(0, '/opt/trn_rl_repo')

import numpy as np
import ml_dtypes

import concourse.bass as bass
import concourse.mybir as mybir
import concourse.tile as tile
from concourse import bacc
from concourse.bass_utils import run_bass_kernel_spmd
from concourse.masks import make_identity

F32 = mybir.dt.float32
F8 = mybir.dt.float8e4
I16 = mybir.dt.int16
F8NP = ml_dtypes.float8_e4m3
BF16 = mybir.dt.bfloat16
DR = mybir.MatmulPerfMode.DoubleRow
MULT = mybir.AluOpType.mult
ADD = mybir.AluOpType.add

NC = 8
TPG = 4               # tensor-parallel group size
P = 128
VOCAB = 50257
EMB = 1024
HID = 4096
NLAB = 4
B, S = 1024, 128
BL = B // NC          # own batch rows per core = 128
GB = B // 2           # group batch rows = 512
FS = HID // TPG       # feature shard per core per layer = 1024
GCH = 8               # stream columns (128 rows each) per dma_gather call
VSPLIT = 32768        # vocab split so indices fit int16
RG = [[0, 1, 2, 3], [4, 5, 6, 7]]

# power-of-2 scale exponents (host pre-scales, device drains fold them out)
SE_EMB = 6            # emb table stored * 2^6
SE_SC = 7             # mask stored = (score/S) * 2^7
SE_W1 = 4
SE_W = 5              # W2..W5
SE_WOUT = 4
SE_POOL = 10          # pooled fp8 stored * 2^10
SE_X = 11             # layer activations stored * 2^11

POOL_DRAIN = 2.0 ** (SE_POOL - SE_EMB - SE_SC)
L1_DRAIN = 2.0 ** (SE_X - SE_POOL - SE_W1)
L_DRAIN = 2.0 ** (SE_X - SE_X - SE_W)
HEAD_DRAIN = 2.0 ** (0 - SE_X - SE_WOUT)

LAST_RESULTS = None   # BassKernelResults of the last run (for test harness)
_PROGRAM_CACHE = None


def _build_program(loc, hic, sim=False):
    """loc/hic: columns (of 128 stream slots) in the low/high token streams.
    Both must be even (DoubleRow pairs stream columns)."""
    assert loc % 2 == 0 and hic % 2 == 0
    nc = bacc.Bacc("TRN2", target_bir_lowering=False, debug=False,
                   enable_asserts=False, num_devices=1 if sim else NC)

    def _collective(kind, op, ins, outs):
        """Real collective, or (sim mode) local DMA copies with roughly the
        same local-HBM traffic so TimelineSim sees equivalent load."""
        if not sim:
            nc.gpsimd.collective_compute(kind, op, replica_groups=RG,
                                         ins=ins, outs=outs)
            return
        in_ap, out_ap = ins[0], outs[0]
        if kind == "AllGather":
            n = in_ap.shape[0]
            for r in range(TPG):
                nc.sync.dma_start(out_ap[r * n:(r + 1) * n], in_ap[:])
        else:  # ReduceScatter
            n = out_ap.shape[0]
            nc.sync.dma_start(out_ap[:], in_ap[0:n])

    # ---------------- I/O ----------------
    tot = loc + hic
    idx_lo = nc.dram_tensor("idx_lo", [P, loc * 8], I16, kind="ExternalInput")
    idx_hi = nc.dram_tensor("idx_hi", [P, hic * 8], I16, kind="ExternalInput")
    masks = nc.dram_tensor("masks", [P, tot, P], F8, kind="ExternalInput")
    emb_lo = nc.dram_tensor("emb_lo", [VSPLIT, EMB], F8, kind="ExternalInput")
    emb_hi = nc.dram_tensor("emb_hi", [VOCAB - VSPLIT, EMB], F8,
                            kind="ExternalInput")
    w1 = nc.dram_tensor("w1", [P, EMB // P, FS], F8, kind="ExternalInput")
    wts = [nc.dram_tensor(f"w{k}", [P, HID // P, FS], F8,
                          kind="ExternalInput") for k in range(2, 6)]
    bs = [nc.dram_tensor(f"b{k}", [P, FS // P], F32, kind="ExternalInput")
          for k in range(1, 6)]
    wout = nc.dram_tensor("wout", [P, FS // P, NLAB], F8,
                          kind="ExternalInput")
    bout = nc.dram_tensor("bout", [P, NLAB], F32, kind="ExternalInput")
    out_loc = nc.dram_tensor("out_loc", [BL, NLAB], F32, kind="ExternalOutput")

    with tile.TileContext(nc) as tc:
        with tc.tile_pool(name="const", bufs=1) as const, \
             tc.tile_pool(name="dram", bufs=1, space="DRAM") as dram:

            # ------------- constants / weight preloads -------------
            itlo = const.tile([P, loc * 8], I16, name="itlo")
            nc.sync.dma_start(itlo[:], idx_lo[:])
            ithi = const.tile([P, hic * 8], I16, name="ithi")
            nc.sync.dma_start(ithi[:], idx_hi[:])
            ident8 = const.tile([P, P], BF16, name="ident8")
            make_identity(nc, ident8[:])

            # warm up the collective stream early with a tiny AllGather so
            # the first real collective doesn't pay the cold-start.
            warm_in = dram.tile([64, NLAB], F8, name="warm_in", tag="warm_in")
            warm_out = dram.tile([TPG * 64, NLAB], F8, name="warm_out",
                                 tag="warm_out",
                                 addr_space="Local")
            _collective("AllGather", mybir.AluOpType.bypass,
                        [warm_in.opt()], [warm_out.opt()])

            w1sb = const.tile([P, EMB // P, FS], F8, name="w1sb")
            nc.sync.dma_start(w1sb[:], w1[:])
            wsbs = []
            for k in range(2, 6):
                t = const.tile([P, HID // P, FS], F8, name=f"wsb{k}",
                               tag=f"wsb{k}")
                nc.sync.dma_start(t[:], wts[k - 2][:])
                wsbs.append(t)
            woutsb = const.tile([P, FS // P, NLAB], F8, name="woutsb")
            nc.sync.dma_start(woutsb[:], wout[:])
            bsb = []
            for k in range(5):
                t = const.tile([P, FS // P], F32, name=f"bsb{k}", tag=f"bsb{k}")
                nc.sync.dma_start(t[:], bs[k][:])
                bsb.append(t)
            boutsb = const.tile([P, NLAB], F32, name="boutsb")
            nc.sync.dma_start(boutsb[:], bout[:])

            pooled = const.tile([P, EMB], BF16, name="pooled")
            pooledT = const.tile([P, EMB // P, P], F8, name="pooledT")
            x5sb = const.tile([P, FS // P, GB], F8, name="x5sb")
            lg = const.tile([P, GB // P, NLAB], F32, name="lg")

            # ============ phase 1: gather + pool ============
            with tc.tile_pool(name="maskp", bufs=3) as maskp, \
                 tc.tile_pool(name="gp", bufs=3) as gp, \
                 tc.tile_pool(name="poolps", bufs=1, space="PSUM") as poolps, \
                 tc.tile_pool(name="smps", bufs=2, space="PSUM") as smps:
                pp0 = poolps.tile([P, 512], F32, name="pp0")
                pp1 = poolps.tile([P, 512], F32, name="pp1")

                chunks = []  # (stream_base, col_in_stream, ncols, table, idx)
                for cols, tbl_ap, it_sb, base in (
                        (loc, emb_lo, itlo, 0), (hic, emb_hi, ithi, loc)):
                    for c0 in range(0, cols, GCH):
                        nch = min(GCH, cols - c0)
                        chunks.append((base, c0, nch, tbl_ap, it_sb))

                for base, c0, nch, tbl_ap, it_sb in chunks:
                    g = gp.tile([P, GCH, EMB], F8, name="g", tag="g")
                    mk = maskp.tile([P, GCH, P], F8, name="mk", tag="mk")
                    nc.sync.dma_start(mk[:, :nch],
                                      masks[:, base + c0:base + c0 + nch, :])
                    nc.gpsimd.dma_gather(
                        out_ap=g[:, :nch], in_ap=tbl_ap[:],
                        idxs_ap=it_sb[:, c0 * 8:(c0 + nch) * 8],
                        num_idxs=nch * P, num_idxs_reg=nch * P,
                        elem_size=EMB)
                    for jp in range(nch // 2):
                        gcp = (base + c0) // 2 + jp   # global column pair
                        st = (gcp == 0)
                        sp = (gcp == tot // 2 - 1)
                        nc.tensor.matmul(
                            pp0[:], lhsT=mk[:, 2 * jp:2 * jp + 2, :],
                            rhs=g[:, 2 * jp:2 * jp + 2, 0:512],
                            start=st, stop=sp, perf_mode=DR)
                        nc.tensor.matmul(
                            pp1[:], lhsT=mk[:, 2 * jp:2 * jp + 2, :],
                            rhs=g[:, 2 * jp:2 * jp + 2, 512:EMB],
                            start=st, stop=sp, perf_mode=DR)

                # drain pooled (fold 2^-(SE_EMB+SE_SC), apply 2^SE_POOL)
                nc.vector.tensor_scalar_mul(pooled[:, 0:512], pp0[:],
                                            POOL_DRAIN)
                nc.vector.tensor_scalar_mul(pooled[:, 512:EMB], pp1[:],
                                            POOL_DRAIN)

                # transpose pooled [b, e] -> pooledT [e_p, eo, b]
                for eo in range(EMB // P):
                    tp = smps.tile([P, P], BF16, name="tp", tag="tp")
                    nc.tensor.transpose(tp[:], pooled[:, eo * P:(eo + 1) * P],
                                        ident8[:])
                    nc.vector.tensor_copy(pooledT[:, eo, :], tp[:])

            pooledT_in = dram.tile([EMB, P], F8, name="pooledT_in",
                                   tag="pooledT_in")
            nc.sync.dma_start(
                pooledT_in[:].rearrange("(eo p) b -> p eo b", p=P), pooledT[:])
            pooledT_full = dram.tile([TPG * EMB, P], F8, name="pooledT_full",
                                     tag="pooledT_full",
                                     addr_space="Local")
            _collective("AllGather", mybir.AluOpType.bypass,
                        [pooledT_in.opt()], [pooledT_full.opt()])

            # chunked AG buffers for L1..L4 outputs (h = feature half)
            xag_in = {}
            xag_out = {}
            for l in range(1, 5):
                for h in range(2):
                    xag_in[(l, h)] = dram.tile(
                        [FS // 2, GB], F8, name=f"xag_in_{l}_{h}",
                        tag=f"xag_in_{l}_{h}")
                    xag_out[(l, h)] = dram.tile(
                        [TPG * (FS // 2), GB], F8, name=f"xag_out_{l}_{h}",
                        tag=f"xag_out_{l}_{h}",
                        addr_space="Local")

            # ============ MLP ============
            with tc.tile_pool(name="mmps", bufs=8, space="PSUM") as mmps, \
                 tc.tile_pool(name="xp1", bufs=4) as xp1, \
                 tc.tile_pool(name="xp", bufs=6) as xp, \
                 tc.tile_pool(name="op", bufs=4) as op:

                # ------------- L1 (K=EMB=1024, rank-subtiled rhs) ---------
                rc1 = []
                for rb in range(TPG):
                    t = xp1.tile([P, EMB // P, P], F8, name=f"rc1_{rb}",
                                 tag="x1")
                    nc.sync.dma_start(
                        t[:], pooledT_full[rb * EMB:(rb + 1) * EMB, :]
                        .rearrange("(ko p) b -> p ko b", p=P))
                    rc1.append(t)
                ps = [mmps.tile([P, GB], F32, name=f"mm1_{m}", tag="mm")
                      for m in range(FS // P)]
                ndk1 = EMB // P // 2   # 4 dk-pairs
                for m in range(FS // P):
                    for dk in range(ndk1):
                        for rb in range(TPG):
                            nc.tensor.matmul(
                                ps[m][:, rb * P:(rb + 1) * P],
                                lhsT=w1sb[:, 2 * dk:2 * dk + 2,
                                          m * P:(m + 1) * P],
                                rhs=rc1[rb][:, 2 * dk:2 * dk + 2, :],
                                start=(dk == 0), stop=(dk == ndk1 - 1),
                                perf_mode=DR)
                for h in range(2):
                    xo = op.tile([P, 4, GB], F8, name=f"xo1_{h}", tag="xo")
                    for mt in range(4):
                        m = 4 * h + mt
                        nc.vector.tensor_scalar(
                            xo[:, mt], ps[m][:], L1_DRAIN,
                            bsb[0][:, m:m + 1], MULT, ADD)
                    nc.sync.dma_start(
                        xag_in[(1, h)][:].rearrange("(mt p) b -> p mt b", p=P),
                        xo[:])
                    _collective("AllGather", mybir.AluOpType.bypass,
                                [xag_in[(1, h)].opt()], [xag_out[(1, h)].opt()])

                # ------------- L2..L5 (K=HID=4096, chunked k-halves) ------
                for li, l in enumerate(range(2, 6)):
                    wsb = wsbs[li]
                    ps = [mmps.tile([P, GB], F32, name=f"mm{l}_{m}", tag="mm")
                          for m in range(FS // P)]
                    for half in range(2):
                        rcs = []
                        for t_i in range(2):
                            t = xp.tile([P, 8, GB], F8, name=f"rc{l}_{half}_{t_i}",
                                        tag="x")
                            nc.sync.dma_start(
                                t[:], xag_out[(l - 1, half)]
                                [t_i * 1024:(t_i + 1) * 1024, :]
                                .rearrange("(ko p) b -> p ko b", p=P))
                            rcs.append(t)
                        for m in range(FS // P):
                            for t_i in range(2):
                                for dp in range(4):
                                    kpg = half * 8 + t_i * 4 + dp
                                    nc.tensor.matmul(
                                        ps[m][:],
                                        lhsT=wsb[:, 2 * kpg:2 * kpg + 2,
                                                 m * P:(m + 1) * P],
                                        rhs=rcs[t_i][:, 2 * dp:2 * dp + 2, :],
                                        start=(kpg == 0),
                                        stop=(kpg == HID // P // 2 - 1),
                                        perf_mode=DR)
                    for h in range(2):
                        if l < 5:
                            xo = op.tile([P, 4, GB], F8, name=f"xo{l}_{h}",
                                         tag="xo")
                        for mt in range(4):
                            m = 4 * h + mt
                            dst = xo[:, mt] if l < 5 else x5sb[:, m]
                            nc.vector.tensor_scalar(
                                dst, ps[m][:], L_DRAIN,
                                bsb[l - 1][:, m:m + 1], MULT, ADD)
                        if l < 5:
                            nc.sync.dma_start(
                                xag_in[(l, h)][:].rearrange(
                                    "(mt p) b -> p mt b", p=P), xo[:])
                            _collective("AllGather", mybir.AluOpType.bypass,
                                        [xag_in[(l, h)].opt()],
                                        [xag_out[(l, h)].opt()])

            # ------------- head: partial logits + RS + log_softmax --------
            with tc.tile_pool(name="headps", bufs=1, space="PSUM") as headps:
                psh = headps.tile([P, GB // P, NLAB], F32, name="psh")
                ndm = FS // P // 2   # 4 dm-pairs
                for bq in range(GB // P):
                    for dm in range(ndm):
                        nc.tensor.matmul(
                            psh[:, bq],
                            lhsT=x5sb[:, 2 * dm:2 * dm + 2,
                                      bq * P:(bq + 1) * P],
                            rhs=woutsb[:, 2 * dm:2 * dm + 2, :],
                            start=(dm == 0), stop=(dm == ndm - 1),
                            perf_mode=DR)
                for bq in range(GB // P):
                    nc.vector.tensor_scalar_mul(lg[:, bq], psh[:, bq],
                                                HEAD_DRAIN)

                rs_in = dram.tile([GB, NLAB], F32, name="rs_in", tag="rs_in")
                nc.sync.dma_start(
                    rs_in[:].rearrange("(q p) l -> p q l", p=P), lg[:])
                rs_out = dram.tile([BL, NLAB], F32, name="rs_out",
                                   tag="rs_out")
                _collective("ReduceScatter", mybir.AluOpType.add,
                            [rs_in.opt()], [rs_out.opt()])

                lgl = const.tile([P, NLAB], F32, name="lgl")
                nc.sync.dma_start(lgl[:], rs_out[:])
                nc.vector.tensor_add(out=lgl[:], in0=lgl[:], in1=boutsb[:])
                negmx = const.tile([P, 1], F32, name="negmx")
                nc.vector.reduce_max(negmx[:], lgl[:],
                                     axis=mybir.AxisListType.X)
                nc.vector.tensor_scalar_mul(negmx[:], negmx[:], -1.0)
                ex = const.tile([P, NLAB], F32, name="ex")
                se = const.tile([P, 1], F32, name="se")
                nc.scalar.activation(ex[:], lgl[:],
                                     mybir.ActivationFunctionType.Exp,
                                     bias=negmx[:, 0:1], scale=1.0,
                                     accum_out=se[:])
                ls = const.tile([P, 1], F32, name="ls")
                nc.scalar.activation(ls[:], se[:],
                                     mybir.ActivationFunctionType.Ln)
                osb = const.tile([P, NLAB], F32, name="osb")
                nc.vector.tensor_scalar(osb[:], lgl[:], negmx[:, 0:1],
                                        ls[:, 0:1], mybir.AluOpType.add,
                                        mybir.AluOpType.subtract)
                nc.sync.dma_start(out_loc[:], osb[:])

    nc.compile()
    return nc


def get_program(loc, hic):
    global _PROGRAM_CACHE
    if _PROGRAM_CACHE is None or _PROGRAM_CACHE[0] != (loc, hic):
        _PROGRAM_CACHE = ((loc, hic), _build_program(loc, hic))
    return _PROGRAM_CACHE[1]


def _even_cols(n):
    c = (n + P - 1) // P
    return c + (c % 2)


def _build_streams(sentence, scores):
    """Per-core deduped low/high token streams + score-mask tensors."""
    cores = []
    for c in range(NC):
        sent = np.asarray(sentence[c * BL:(c + 1) * BL]).astype(np.int64)
        uniq, inv = np.unique(sent.ravel(), return_inverse=True)
        nlo = int((uniq < VSPLIT).sum())
        cores.append((uniq, inv, nlo))

    loc = max(_even_cols(t[2]) for t in cores)
    hic = max(_even_cols(len(t[0]) - t[2]) for t in cores)

    per_core = []
    for c in range(NC):
        uniq, inv, nlo = cores[c]
        nhi = len(uniq) - nlo
        sc = (np.asarray(scores[c * BL:(c + 1) * BL], np.float32)
              / np.float32(S) * np.float32(2.0 ** SE_SC))

        # stream position of each distinct token
        pos_row = np.empty(len(uniq), np.int64)
        pos_col = np.empty(len(uniq), np.int64)
        lo_idx = np.arange(nlo)
        pos_row[:nlo] = lo_idx % P
        pos_col[:nlo] = lo_idx // P
        hi_idx = np.arange(nhi)
        pos_row[nlo:] = hi_idx % P
        pos_col[nlo:] = loc + hi_idx // P

        mask = np.zeros((P, loc + hic, P), np.float32)
        brow = np.repeat(np.arange(BL), S)
        np.add.at(mask, (pos_row[inv], pos_col[inv], brow), sc.ravel())

        idx_arrs = []
        for toks, cols in ((uniq[:nlo], loc), (uniq[nlo:] - VSPLIT, hic)):
            stream = np.zeros(cols * P, np.int16)
            stream[:len(toks)] = toks.astype(np.int16)
            idx = np.tile(stream.reshape(cols * 8, 16).T, (8, 1))
            idx_arrs.append(np.ascontiguousarray(idx.astype(np.int16)))
        per_core.append({
            "idx_lo": idx_arrs[0], "idx_hi": idx_arrs[1],
            "masks": np.ascontiguousarray(mask).astype(F8NP),
        })
    return loc, hic, per_core


def prep_in_maps(sentence, scores, emb, W1, b1, W2, b2, W3, b3, W4, b4, W5,
                 b5, Wout, bout):
    loc, hic, streams = _build_streams(sentence, scores)

    emb_h = (np.asarray(emb, np.float32) * 2.0 ** SE_EMB).astype(F8NP)
    emb_lo_h = np.ascontiguousarray(emb_h[:VSPLIT])
    emb_hi_h = np.ascontiguousarray(emb_h[VSPLIT:])

    # input-dim permutation matching the chunked AG row order:
    # [r0 f0-511, r1 f0-511, ..., r3 f0-511, r0 f512-1023, ...]
    perm = np.concatenate([r * FS + h * (FS // 2) + np.arange(FS // 2)
                           for h in range(2) for r in range(TPG)])

    def pack(wt, nko):   # [K, M] -> [P, nko, M]
        return np.ascontiguousarray(
            wt.reshape(nko, P, wt.shape[1]).transpose(1, 0, 2))

    w1t = np.asarray(W1, np.float32).T * 2.0 ** SE_W1     # [EMB, HID]
    wlt = [np.asarray(w, np.float32).T * 2.0 ** SE_W
           for w in (W2, W3, W4, W5)]                      # [HID, HID]
    woutt = np.asarray(Wout, np.float32).T * 2.0 ** SE_WOUT  # [HID, 4]
    bss = [np.asarray(b, np.float32) * 2.0 ** SE_X
           for b in (b1, b2, b3, b4, b5)]
    bout_h = np.tile(np.asarray(bout, np.float32)[None, :], (P, 1))

    by_rank = []
    for rt in range(TPG):
        fsl = slice(rt * FS, (rt + 1) * FS)
        m = {
            "w1": pack(w1t[:, fsl].astype(F8NP), EMB // P),
            "wout": pack(woutt[fsl].astype(F8NP), FS // P),
        }
        for k, wt in zip(range(2, 6), wlt):
            m[f"w{k}"] = pack(wt[perm][:, fsl].astype(F8NP), HID // P)
        for k, b in zip(range(1, 6), bss):
            m[f"b{k}"] = np.ascontiguousarray(
                b[fsl].reshape(FS // P, P).T.astype(np.float32))
        by_rank.append(m)

    in_maps = []
    for c in range(NC):
        m = {
            "emb_lo": emb_lo_h,
            "emb_hi": emb_hi_h,
            "bout": bout_h,
        }
        m.update(by_rank[c % TPG])
        m.update(streams[c])
        in_maps.append(m)
    return (loc, hic), in_maps


def kernel(sentence, scores, emb, W1, b1, W2, b2, W3, b3, W4, b4, W5, b5,
           Wout, bout):
    global LAST_RESULTS
    (loc, hic), in_maps = prep_in_maps(sentence, scores, emb, W1, b1, W2, b2,
                                       W3, b3, W4, b4, W5, b5, Wout, bout)
    nc = get_program(loc, hic)
    res = run_bass_kernel_spmd(nc, in_maps, core_ids=list(range(NC)))
    LAST_RESULTS = res
    out = np.concatenate([res.results[c]["out_loc"] for c in range(NC)],
                         axis=0)
    return out.astype(np.float32)
